# revision 28
# baseline (speedup 1.0000x reference)
"""CondTransport kernel for Trainium2 (8 NeuronCores, row-parallel).

Math: Z = Y_mean + Y_var + k_mean @ V_mean + k_var @ V_var, where
k(X, X) = exp(-||x_i - x_j||^2 / (2 l^2)) are 8192x8192 RBF Gram matrices
over X_mean = [X_mu, Y_mean+Y_var] (96-d, l=7) and
X_var = [X_mu, 0.01*flip(Y_eta), Y_mean+Y_var] (160-d, l=9).

S-form factorization used on device:
  k[i,j] = e_i * e_j * exp(G_ij / l^2),   G = X X^T,  e_i = exp(-rn_i/(2 l^2))
so  Z[i] = e_i * sum_j exp(G_ij/l^2) (e_j V[j]).  The e_j factor is folded
into the V weights on the host, the e_i factor into the host epilogue, and
the device computes pure exp(G * scale + const).

Engine split (the v1 steady state was ScalarE-bound at 1087 ns/jt = 64
gapless ACTIVATE exps; v2+ rebalances):
  - em tiles are produced by TWO engines: ScalarE computes exact ACT-exp
    for 2/3 of j-tiles; the DVE computes the other 1/3 via a Schraudolph
    int16-bitcast exp (bits16(em) ~= A*G + B, one fp32-in tensor_scalar
    per tile, max value error ~3.1%, end-to-end contribution <1e-3).
  - VectorE derives the VAR map from em with the log-domain bitcast pow:
    bits16(em^c) ~= c*bits16(em) + (1-c)*15360 + B_ADJ  (c = 49/81), one
    int16 4x-mode tensor_scalar per j-tile (per-tile so var-Z(jt) pairs
    with mean-Z(jt) and nothing drains unpaired at the end).
  - PE: every matmul is M=64 into a PSUM partition half (Gram j-halves
    0:64/64:128 of pg; mean-Z 0:64 / var-Z 64:128 of pz), alternating the
    two PE column-group pairs, so consecutive ops execute CONCURRENTLY in
    disjoint column groups and every LDWEIGHTS hides under the other
    group's running matmul.  Steady state ~867 ns/j-tile vs the 854ns
    4x512-cycle floor (the M=128 Gram form paid ~240ns/jt of exposed
    weight-load latency).
  - j-ROTATION: per-core inputs are host-rolled so this core's own 1024
    rows are j-tiles 0..7; the Gram moving operand IS sxm[:, 0:1024] (no
    duplicate own-rows DMA, shortest critical path to the first Gram).
  - Startup: memsets + ACT-table-load + 5 warm-up matmuls cover the
    ~7.9..10us window until the first operands land (the NEFF preamble
    alone is ~6.4us; the PE p-state reaches 2.4GHz ~10-14us after first
    activity).  DMAs issue ONLY from sync/gpsimd (+1 early scalar DMA):
    descriptor-gen costs the issuing engine ~0.8us per DMA, so Scalar/
    Vector must stay DMA-free for the exp work.
  - Tail: pz is split per i-half into single-bank PSUM tiles; each half's
    fp32->fp16 copy (ScalarE h0 / VectorE h1) fires the moment its last
    matmul stops, then one out-DMA per queue.  The last two j-tiles'
    em/ev are produced in halves on both engines to shorten the end chain
    (em -> ev -> var-Z -> copy -> out-DMA).  Writeback in fp16 (fp16
    rounding of Z is ~5e-4 relative, negligible).

Sharding: rows i split 1024-per-core (8 cores); each core holds full X.
"""

import numpy as np

N = 8192
DX = 32
DY = 64
NCORES = 8
ROWS = N // NCORES          # 1024 rows per core
NJT = N // 128              # 64 j-tiles
DM = DX + DY                # 96 mean features
L_MEAN = 7.0
L_VAR = 9.0
VAR_EPS = 0.01

C_POW = (L_MEAN * L_MEAN) / (L_VAR * L_VAR)      # 49/81
SHIFT = float(np.log(4.0))                        # em = exp(G/49 - ln4)
B_POW = (1.0 - C_POW) * 15360.0 - 18.0            # fastpow offset (tuned)

LN2 = float(np.log(2.0))
INV2LM = 1.0 / (L_MEAN * L_MEAN)
A_EXP = 1024.0 * INV2LM / LN2                     # Schraudolph slope
B_EXP = 1024.0 * (15.0 - SHIFT / LN2) - 46.0      # Schraudolph offset (tuned)

_CACHE = {}


def _build_nc():
    import concourse.mybir as mybir
    import concourse.tile as tile
    from concourse import bacc

    f32 = mybir.dt.float32
    f16 = mybir.dt.float16
    i16 = mybir.dt.int16
    Exp = mybir.ActivationFunctionType.Exp
    Alu = mybir.AluOpType

    nc = bacc.Bacc(None, target_bir_lowering=False)

    # Per-core inputs are j-ROTATED by the core's row offset (host-side
    # np.roll), so each core's own rows ARE j-tiles 0..7: the Gram moving
    # operand is sxm[:, 0:ROWS] and no separate own-rows DMA is needed.
    xmT_all = nc.declare_dram_parameter("xmT_all", [DM, N], f16, isOutput=False)
    vm = nc.declare_dram_parameter("vm", [128, NJT * DY], f16, isOutput=False)
    vv = nc.declare_dram_parameter("vv", [128, NJT * DY], f16, isOutput=False)
    zT = nc.declare_dram_parameter("zT", [128, ROWS], f16, isOutput=True)

    with tile.TileContext(nc) as tc:
        with (
            tc.tile_pool(name="data", bufs=1) as data,
            tc.tile_pool(name="etiles", bufs=3) as etiles,
            tc.tile_pool(name="psg", bufs=3, space="PSUM") as psg,
            tc.tile_pool(name="psz", bufs=1, space="PSUM") as psz,
        ):
            sxm = data.tile([DM, N], f16)
            svm = data.tile([128, NJT * DY], f16)
            svv = data.tile([128, NJT * DY], f16)
            warm = data.tile([DM, 512], f16)
            bias_t = data.tile([128, 1], f32)
            dummy = data.tile([128, 1], f16)

            H = ROWS // 2  # 512-wide halves (PSUM bank / moving-op limit)
            # Z^T accumulators, ONE PSUM BANK PER i-HALF (partitions 0:64
            # mean, 64:128 var).  Separate tiles per half so each half's
            # PSUM->SBUF copy fires as soon as ITS last matmul stops (a
            # single [128,1024] tile serialized the copies behind the very
            # last var matmul).
            pzh = [psz.tile([128, H], f32, name=f"pz{h}") for h in range(2)]

            # --- t=0 work that runs during the DMA window -----------------
            # memsets on GpSimd (starts immediately), ACT table load early,
            # and 5 serial warm-up matmuls that keep the PE busy (p-state
            # ramping toward 2.4 GHz) until the first real Gram's operands
            # land (~10.3us; engine-start alone is ~7.2us).
            nc.gpsimd.memset(warm, 0.0)
            nc.gpsimd.memset(bias_t, -SHIFT)
            nc.scalar.activation(dummy, bias_t, Exp, bias=bias_t[:, :], scale=1.0)
            for w in range(5):
                nc.tensor.matmul(
                    pzh[0][:, :], warm[:, 0:128], warm[:, :], start=True, stop=True,
                )

            # --- DMA-in.  Head: the own block [0:1024] (j-tiles 0..7 AND
            # the Gram moving operand, thanks to the rotation) split across
            # sync/gpsimd; j-tiles 8..11 on ONE early scalar-queue DMA (the
            # Scalar engine must otherwise stay DMA-free: queued DMA issue
            # ahead of the EXPs delays them by ~0.8us each).
            nc.sync.dma_start(out=sxm[:, 0:512], in_=xmT_all[:, 0:512])
            nc.gpsimd.dma_start(out=sxm[:, 512:1024], in_=xmT_all[:, 512:1024])
            nc.scalar.dma_start(out=sxm[:, 1024:1536], in_=xmT_all[:, 1024:1536])
            # Rest interleaved in consumption order (V chunk k feeds the Z
            # matmuls of j-tiles 8k..8k+7; X feeds Grams ~3 j-tiles ahead).
            VCH = (NJT * DY) // 8
            items = []
            for k in range(8):
                items.append((svm, vm, slice(k * VCH, (k + 1) * VCH)))
                items.append((svv, vv, slice(k * VCH, (k + 1) * VCH)))
                cs, ce = 1536 + k * 1024, min(1536 + (k + 1) * 1024, N)
                if cs < N:
                    items.append((sxm, xmT_all, slice(cs, ce)))
            for i, (dst, src, sl) in enumerate(items):
                q = nc.sync if i % 2 == 0 else nc.gpsimd
                q.dma_start(out=dst[:, sl], in_=src[:, sl])

            def emit_gram(jt):
                """G(jt) [128 j x ROWS i] fp32 into a rotating PSUM slot.

                Emitted as FOUR M=64 matmuls alternating between the two PE
                column-group pairs (out partitions 0:64 vs 64:128), so every
                weight load lands in one group while the other group's matmul
                streams -- no exposed LDWEIGHTS latency (the full-width M=128
                form paid ~240ns/jt of weight-load stalls).
                """
                jb = jt * 128
                pg = psg.tile([128, ROWS], f32, tag="pg", bufs=3, name=f"pg{jt}")
                for h in range(2):
                    hs = slice(h * H, (h + 1) * H)
                    nc.tensor.matmul(
                        pg[0:64, hs], sxm[:, jb : jb + 64], sxm[:, hs],
                        start=True, stop=True,
                    )
                    nc.tensor.matmul(
                        pg[64:128, hs], sxm[:, jb + 64 : jb + 128], sxm[:, hs],
                        start=True, stop=True,
                    )
                return pg

            # Main loop.  em/ev tiles span 4 j-tiles (SBUF layout only); the
            # ev fastpow runs PER j-tile so var-Z(jt) pairs with mean-Z(jt)
            # in the same slot -- zero end-of-loop drain (the 4-tile-grouped
            # ev lagged ~2 groups and left ~3.3us of unpaired var-Z at the
            # end, which also blocked the PSUM->SBUF copies).
            GRP = 4
            grams = [emit_gram(0), emit_gram(1), emit_gram(2)]
            em_t = ev_t = None
            ev_tiles = {}
            pending = []  # deferred var-Z j-tiles, interleaved with later mean-Z

            def emit_var_z(jv, h):
                """var-Z half (PE column groups 2-3, concurrent with mean-Z)."""
                vbv = slice(jv * DY, (jv + 1) * DY)
                ov = (jv % GRP) * ROWS
                nc.tensor.matmul(
                    pzh[h][64:128, :], svv[:, vbv],
                    ev_tiles[jv // GRP][:, ov + h * H : ov + (h + 1) * H],
                    start=(jv == 0), stop=(jv == NJT - 1),
                )

            for jt in range(NJT):
                g, o = jt // GRP, (jt % GRP) * ROWS
                if jt % GRP == 0:
                    em_t = etiles.tile([128, GRP * ROWS], f16, tag="em", name=f"em{g}")
                    ev_t = etiles.tile([128, GRP * ROWS], f16, tag="ev", name=f"ev{g}")
                    ev_tiles[g] = ev_t
                pg = grams[jt]
                if jt >= NJT - 2:
                    # Last two j-tiles sit on the kernel's end chain
                    # (em -> ev -> var-Z -> copy -> out-DMA): produce their
                    # em/ev in HALVES on both engines concurrently.
                    nc.scalar.activation(
                        em_t[:, o : o + H], pg[:, 0:H], Exp,
                        bias=bias_t[:, :], scale=INV2LM,
                    )
                    nc.vector.tensor_scalar(
                        out=em_t[:, o + H : o + ROWS].bitcast(i16),
                        in0=pg[:, H:ROWS],
                        scalar1=A_EXP, scalar2=B_EXP,
                        op0=Alu.mult, op1=Alu.add,
                    )
                    for fh in (1, 0):
                        fs = slice(o + fh * H, o + (fh + 1) * H)
                        nc.vector.tensor_scalar(
                            out=ev_t[:, fs].bitcast(i16),
                            in0=em_t[:, fs].bitcast(i16),
                            scalar1=C_POW, scalar2=B_POW,
                            op0=Alu.mult, op1=Alu.add,
                        )
                    pending.extend(((jt, 0), (jt, 1)))
                else:
                    if jt % 3 == 1:
                        # DVE Schraudolph exp: bits16(em) = A*G + B (fp32-in)
                        nc.vector.tensor_scalar(
                            out=em_t[:, o : o + ROWS].bitcast(i16),
                            in0=pg[:, :],
                            scalar1=A_EXP, scalar2=B_EXP,
                            op0=Alu.mult, op1=Alu.add,
                        )
                    else:
                        nc.scalar.activation(
                            em_t[:, o : o + ROWS], pg[:, :], Exp,
                            bias=bias_t[:, :], scale=INV2LM,
                        )
                    # fastpow: ev = em^(49/81) via int16 bitcast, one 4x-mode
                    # op per PAIR of j-tiles (the per-jt form saturated the
                    # DVE at ~90% busy and its jitter stalled the Gram's
                    # PSUM rotation ~350ns every ~6 j-tiles).
                    if jt % 2 == 1:
                        po = (jt % GRP - 1) * ROWS
                        nc.vector.tensor_scalar(
                            out=ev_t[:, po : po + 2 * ROWS].bitcast(i16),
                            in0=em_t[:, po : po + 2 * ROWS].bitcast(i16),
                            scalar1=C_POW, scalar2=B_POW,
                            op0=Alu.mult, op1=Alu.add,
                        )
                        pending.extend(
                            ((jt - 1, 0), (jt - 1, 1), (jt, 0), (jt, 1))
                        )
                if jt + 3 < NJT:
                    grams.append(emit_gram(jt + 3))

                # mean-Z for this j-tile (PE column groups 0-1), each half
                # followed by a pending var-Z half so the two Z streams sit
                # adjacent in the PE queue and overlap in disjoint col groups.
                vb = slice(jt * DY, (jt + 1) * DY)
                for h in range(2):
                    nc.tensor.matmul(
                        pzh[h][0:64, :], svm[:, vb],
                        em_t[:, o + h * H : o + (h + 1) * H],
                        start=(jt == 0), stop=(jt == NJT - 1),
                    )
                    if pending:
                        emit_var_z(*pending.pop(0))
            while pending:
                emit_var_z(*pending.pop(0))

            # Tail: one [128, 512] fp32->fp16 copy per i-half (ScalarE takes
            # h0 the moment pzh[0]'s last matmul stops, VectorE h1), then one
            # out-DMA per hardware queue.
            szT = data.tile([128, ROWS], f16)
            for h, (ceng, q) in enumerate(
                [(nc.scalar, nc.sync), (nc.vector, nc.gpsimd)]
            ):
                hs = slice(h * H, (h + 1) * H)
                if ceng is nc.scalar:
                    ceng.copy(szT[:, hs], pzh[h][:, :])
                else:
                    ceng.tensor_copy(szT[:, hs], pzh[h][:, :])
                q.dma_start(out=zT[:, hs], in_=szT[:, hs])

    nc.finalize()
    return nc


def _get_nc():
    if "nc" not in _CACHE:
        _CACHE["nc"] = _build_nc()
    return _CACHE["nc"]


def prep_inputs(X_mu, Y_eta, Y_mean, Y_var, V_mean, V_var):
    """Host-side prep: layouts, norms, prescaled V.  Returns (in_maps, e_m, e_v, ymv)."""
    X_mu, Y_eta, Y_mean, Y_var, V_mean, V_var = (
        np.asarray(a, dtype=np.float32)
        for a in (X_mu, Y_eta, Y_mean, Y_var, V_mean, V_var)
    )
    ymv = (Y_mean.astype(np.float64) + Y_var.astype(np.float64)).astype(np.float32)
    # fp16 features: PE products of fp16 inputs are exact in the fp32 PSUM
    # accumulation, so deriving the row norms from the QUANTIZED features
    # keeps k = e_i e_j exp(G/l^2) consistent.
    Xm = np.concatenate([X_mu, ymv], axis=1).astype(np.float32).astype(np.float16)
    f = (VAR_EPS * Y_eta[::-1].astype(np.float64)).astype(np.float16)  # [N, 64]

    rn_m = np.sum(Xm.astype(np.float64) ** 2, axis=1)                # [N]
    rn_v = rn_m + np.sum(f.astype(np.float64) ** 2, axis=1)

    e_m = np.exp(-rn_m / (2.0 * L_MEAN * L_MEAN))                    # fp64 [N]
    e_v = np.exp(-rn_v / (2.0 * L_VAR * L_VAR))

    # prescaled weights: V'' = e_j * V[j]
    Vm_p = (e_m[:, None] * V_mean.astype(np.float64)).astype(np.float16)
    Vv_p = (e_v[:, None] * V_var.astype(np.float64)).astype(np.float16)

    def v_tiles(Vp):
        # [128, jt*64+d] tile layout
        return np.ascontiguousarray(
            Vp.reshape(NJT, 128, DY).transpose(1, 0, 2).reshape(128, NJT * DY)
        )

    in_maps = []
    for c in range(NCORES):
        # j-rotation: roll rows so this core's own rows are j-tiles 0..7
        # (the kernel uses sxm[:, 0:ROWS] as both the Gram moving operand
        # and the first 8 j-tiles' weights -- one DMA instead of two).
        r = c * ROWS
        Xc = np.roll(Xm, -r, axis=0)
        in_maps.append(dict(
            xmT_all=np.ascontiguousarray(Xc.T),
            vm=v_tiles(np.roll(Vm_p, -r, axis=0)),
            vv=v_tiles(np.roll(Vv_p, -r, axis=0)),
        ))
    return in_maps, e_m, e_v, ymv


def postprocess(results, e_m, e_v, ymv):
    """Gather per-core z^T outputs and apply the e_i row factors + Y terms."""
    out = ymv.astype(np.float64).copy()
    sm = 4.0                      # undo the -ln4 shift in em
    sv = 4.0 ** C_POW             # undo the -c*ln4 shift in ev
    for c in range(NCORES):
        rs = slice(c * ROWS, (c + 1) * ROWS)
        zt = results[c]["zT"].astype(np.float64)  # [128, ROWS]
        out[rs] += (sm * e_m[rs])[:, None] * zt[0:64].T
        out[rs] += (sv * e_v[rs])[:, None] * zt[64:128].T
    return out.astype(np.float32)


def kernel(X_mu, Y_eta, Y_mean, Y_var, V_mean, V_var):
    from concourse.bass_utils import run_bass_kernel_spmd

    nc = _get_nc()
    in_maps, e_m, e_v, ymv = prep_inputs(X_mu, Y_eta, Y_mean, Y_var, V_mean, V_var)
    res = run_bass_kernel_spmd(nc, in_maps, core_ids=list(range(NCORES)))
    return postprocess(res.results, e_m, e_v, ymv)


# revision 29
# speedup vs baseline: 1.0179x; 1.0179x over previous
"""CondTransport kernel for Trainium2 (8 NeuronCores, row-parallel).

Math: Z = Y_mean + Y_var + k_mean @ V_mean + k_var @ V_var, where
k(X, X) = exp(-||x_i - x_j||^2 / (2 l^2)) are 8192x8192 RBF Gram matrices
over X_mean = [X_mu, Y_mean+Y_var] (96-d, l=7) and
X_var = [X_mu, 0.01*flip(Y_eta), Y_mean+Y_var] (160-d, l=9).

S-form factorization used on device:
  k[i,j] = e_i * e_j * exp(G_ij / l^2),   G = X X^T,  e_i = exp(-rn_i/(2 l^2))
so  Z[i] = e_i * sum_j exp(G_ij/l^2) (e_j V[j]).  The e_j factor is folded
into the V weights on the host, the e_i factor into the host epilogue, and
the device computes pure exp(G * scale + const).

Engine split (the v1 steady state was ScalarE-bound at 1087 ns/jt = 64
gapless ACTIVATE exps; v2+ rebalances):
  - em tiles are produced by TWO engines: ScalarE computes exact ACT-exp
    for 2/3 of j-tiles; the DVE computes the other 1/3 via a Schraudolph
    int16-bitcast exp (bits16(em) ~= A*G + B, one fp32-in tensor_scalar
    per tile, max value error ~3.1%, end-to-end contribution <1e-3).
  - VectorE derives the VAR map from em with the log-domain bitcast pow:
    bits16(em^c) ~= c*bits16(em) + (1-c)*15360 + B_ADJ  (c = 49/81), one
    int16 4x-mode tensor_scalar per j-tile (per-tile so var-Z(jt) pairs
    with mean-Z(jt) and nothing drains unpaired at the end).
  - PE: every matmul is M=64 into a PSUM partition half (Gram j-halves
    0:64/64:128 of pg; mean-Z 0:64 / var-Z 64:128 of pz), alternating the
    two PE column-group pairs, so consecutive ops execute CONCURRENTLY in
    disjoint column groups and every LDWEIGHTS hides under the other
    group's running matmul.  Steady state ~867 ns/j-tile vs the 854ns
    4x512-cycle floor (the M=128 Gram form paid ~240ns/jt of exposed
    weight-load latency).
  - j-ROTATION: per-core inputs are host-rolled so this core's own 1024
    rows are j-tiles 0..7; the Gram moving operand IS sxm[:, 0:1024] (no
    duplicate own-rows DMA, shortest critical path to the first Gram).
  - Startup: memsets + ACT-table-load + 5 warm-up matmuls cover the
    ~7.9..10us window until the first operands land (the NEFF preamble
    alone is ~6.4us; the PE p-state reaches 2.4GHz ~10-14us after first
    activity).  DMAs issue ONLY from sync/gpsimd (+1 early scalar DMA):
    descriptor-gen costs the issuing engine ~0.8us per DMA, so Scalar/
    Vector must stay DMA-free for the exp work.
  - Tail: pz is split per i-half into single-bank PSUM tiles; each half's
    fp32->fp16 copy (ScalarE h0 / VectorE h1) fires the moment its last
    matmul stops, then one out-DMA per queue.  The last two j-tiles'
    em/ev are produced in halves on both engines to shorten the end chain
    (em -> ev -> var-Z -> copy -> out-DMA).  Writeback in fp16 (fp16
    rounding of Z is ~5e-4 relative, negligible).

Sharding: rows i split 1024-per-core (8 cores); each core holds full X.
"""

import numpy as np

N = 8192
DX = 32
DY = 64
NCORES = 8
ROWS = N // NCORES          # 1024 rows per core
NJT = N // 128              # 64 j-tiles
DM = DX + DY                # 96 mean features
L_MEAN = 7.0
L_VAR = 9.0
VAR_EPS = 0.01

C_POW = (L_MEAN * L_MEAN) / (L_VAR * L_VAR)      # 49/81
SHIFT = float(np.log(4.0))                        # em = exp(G/49 - ln4)
B_POW = (1.0 - C_POW) * 15360.0 - 18.0            # fastpow offset (tuned)

LN2 = float(np.log(2.0))
INV2LM = 1.0 / (L_MEAN * L_MEAN)
A_EXP = 1024.0 * INV2LM / LN2                     # Schraudolph slope
B_EXP = 1024.0 * (15.0 - SHIFT / LN2) - 46.0      # Schraudolph offset (tuned)

_CACHE = {}


def _build_nc():
    import concourse.mybir as mybir
    import concourse.tile as tile
    from concourse import bacc

    f32 = mybir.dt.float32
    f16 = mybir.dt.float16
    i16 = mybir.dt.int16
    Exp = mybir.ActivationFunctionType.Exp
    Alu = mybir.AluOpType

    nc = bacc.Bacc(None, target_bir_lowering=False)

    # Per-core inputs are j-ROTATED by the core's row offset (host-side
    # np.roll), so each core's own rows ARE j-tiles 0..7: the Gram moving
    # operand is sxm[:, 0:ROWS] and no separate own-rows DMA is needed.
    xmT_all = nc.declare_dram_parameter("xmT_all", [DM, N], f16, isOutput=False)
    vm = nc.declare_dram_parameter("vm", [128, NJT * DY], f16, isOutput=False)
    vv = nc.declare_dram_parameter("vv", [128, NJT * DY], f16, isOutput=False)
    zT = nc.declare_dram_parameter("zT", [128, ROWS], f16, isOutput=True)

    with tile.TileContext(nc) as tc:
        with (
            tc.tile_pool(name="data", bufs=1) as data,
            tc.tile_pool(name="etiles", bufs=3) as etiles,
            tc.tile_pool(name="psg", bufs=3, space="PSUM") as psg,
            tc.tile_pool(name="psz", bufs=1, space="PSUM") as psz,
        ):
            sxm = data.tile([DM, N], f16)
            svm = data.tile([128, NJT * DY], f16)
            svv = data.tile([128, NJT * DY], f16)
            warm = data.tile([DM, 512], f16)
            bias_t = data.tile([128, 1], f32)
            dummy = data.tile([128, 1], f16)

            H = ROWS // 2  # 512-wide halves (PSUM bank / moving-op limit)
            # Z^T accumulators, ONE PSUM BANK PER i-HALF (partitions 0:64
            # mean, 64:128 var).  Separate tiles per half so each half's
            # PSUM->SBUF copy fires as soon as ITS last matmul stops (a
            # single [128,1024] tile serialized the copies behind the very
            # last var matmul).
            pzh = [psz.tile([128, H], f32, name=f"pz{h}") for h in range(2)]

            # --- t=0 work that runs during the DMA window -----------------
            # memsets on GpSimd (starts immediately), ACT table load early,
            # and 5 serial warm-up matmuls that keep the PE busy (p-state
            # ramping toward 2.4 GHz) until the first real Gram's operands
            # land (~10.3us; engine-start alone is ~7.2us).
            nc.gpsimd.memset(warm, 0.0)
            nc.gpsimd.memset(bias_t, -SHIFT)
            nc.scalar.activation(dummy, bias_t, Exp, bias=bias_t[:, :], scale=1.0)
            for w in range(5):
                nc.tensor.matmul(
                    pzh[0][:, :], warm[:, 0:128], warm[:, :], start=True, stop=True,
                )

            # --- DMA-in.  Head: the own block [0:1024] (j-tiles 0..7 AND
            # the Gram moving operand, thanks to the rotation) split across
            # sync/gpsimd; j-tiles 8..11 on ONE early scalar-queue DMA (the
            # Scalar engine must otherwise stay DMA-free: queued DMA issue
            # ahead of the EXPs delays them by ~0.8us each).
            nc.sync.dma_start(out=sxm[:, 0:512], in_=xmT_all[:, 0:512])
            nc.gpsimd.dma_start(out=sxm[:, 512:1024], in_=xmT_all[:, 512:1024])
            nc.scalar.dma_start(out=sxm[:, 1024:1536], in_=xmT_all[:, 1024:1536])
            # Rest interleaved in consumption order (V chunk k feeds the Z
            # matmuls of j-tiles 8k..8k+7; X feeds Grams ~3 j-tiles ahead).
            VCH = (NJT * DY) // 8
            items = []
            for k in range(8):
                items.append((svm, vm, slice(k * VCH, (k + 1) * VCH)))
                items.append((svv, vv, slice(k * VCH, (k + 1) * VCH)))
                cs, ce = 1536 + k * 1024, min(1536 + (k + 1) * 1024, N)
                if cs < N:
                    items.append((sxm, xmT_all, slice(cs, ce)))
            for i, (dst, src, sl) in enumerate(items):
                q = nc.sync if i % 2 == 0 else nc.gpsimd
                q.dma_start(out=dst[:, sl], in_=src[:, sl])

            def emit_gram(jt):
                """G(jt) [128 j x ROWS i] fp32 into a rotating PSUM slot.

                Emitted as FOUR M=64 matmuls alternating between the two PE
                column-group pairs (out partitions 0:64 vs 64:128), so every
                weight load lands in one group while the other group's matmul
                streams -- no exposed LDWEIGHTS latency (the full-width M=128
                form paid ~240ns/jt of weight-load stalls).
                """
                jb = jt * 128
                pg = psg.tile([128, ROWS], f32, tag="pg", bufs=3, name=f"pg{jt}")
                for h in range(2):
                    hs = slice(h * H, (h + 1) * H)
                    nc.tensor.matmul(
                        pg[0:64, hs], sxm[:, jb : jb + 64], sxm[:, hs],
                        start=True, stop=True,
                    )
                    nc.tensor.matmul(
                        pg[64:128, hs], sxm[:, jb + 64 : jb + 128], sxm[:, hs],
                        start=True, stop=True,
                    )
                return pg

            # Main loop.  em/ev tiles span 4 j-tiles (SBUF layout only); the
            # ev fastpow runs PER j-tile so var-Z(jt) pairs with mean-Z(jt)
            # in the same slot -- zero end-of-loop drain (the 4-tile-grouped
            # ev lagged ~2 groups and left ~3.3us of unpaired var-Z at the
            # end, which also blocked the PSUM->SBUF copies).
            GRP = 4
            grams = [emit_gram(0), emit_gram(1), emit_gram(2)]
            em_t = ev_t = None
            ev_tiles = {}
            pending = []  # deferred var-Z j-tiles, interleaved with later mean-Z

            def emit_var_z(jv, h):
                """var-Z half (PE column groups 2-3, concurrent with mean-Z)."""
                vbv = slice(jv * DY, (jv + 1) * DY)
                ov = (jv % GRP) * ROWS
                nc.tensor.matmul(
                    pzh[h][64:128, :], svv[:, vbv],
                    ev_tiles[jv // GRP][:, ov + h * H : ov + (h + 1) * H],
                    start=(jv == 0), stop=(jv == NJT - 1),
                )

            for jt in range(NJT):
                g, o = jt // GRP, (jt % GRP) * ROWS
                if jt % GRP == 0:
                    em_t = etiles.tile([128, GRP * ROWS], f16, tag="em", name=f"em{g}")
                    ev_t = etiles.tile([128, GRP * ROWS], f16, tag="ev", name=f"ev{g}")
                    ev_tiles[g] = ev_t
                pg = grams[jt]
                if jt >= NJT - 2:
                    # Last two j-tiles sit on the kernel's end chain
                    # (em -> ev -> var-Z -> copy -> out-DMA): produce their
                    # em/ev in HALVES on both engines concurrently.
                    nc.scalar.activation(
                        em_t[:, o : o + H], pg[:, 0:H], Exp,
                        bias=bias_t[:, :], scale=INV2LM,
                    )
                    nc.vector.tensor_scalar(
                        out=em_t[:, o + H : o + ROWS].bitcast(i16),
                        in0=pg[:, H:ROWS],
                        scalar1=A_EXP, scalar2=B_EXP,
                        op0=Alu.mult, op1=Alu.add,
                    )
                    for fh in (1, 0):
                        fs = slice(o + fh * H, o + (fh + 1) * H)
                        nc.vector.tensor_scalar(
                            out=ev_t[:, fs].bitcast(i16),
                            in0=em_t[:, fs].bitcast(i16),
                            scalar1=C_POW, scalar2=B_POW,
                            op0=Alu.mult, op1=Alu.add,
                        )
                    pending.extend(((jt, 0), (jt, 1)))
                else:
                    if jt % 3 == 1:
                        # DVE Schraudolph exp: bits16(em) = A*G + B (fp32-in)
                        nc.vector.tensor_scalar(
                            out=em_t[:, o : o + ROWS].bitcast(i16),
                            in0=pg[:, :],
                            scalar1=A_EXP, scalar2=B_EXP,
                            op0=Alu.mult, op1=Alu.add,
                        )
                    else:
                        nc.scalar.activation(
                            em_t[:, o : o + ROWS], pg[:, :], Exp,
                            bias=bias_t[:, :], scale=INV2LM,
                        )
                    # fastpow: ev = em^(49/81) via int16 bitcast (DVE 4x
                    # mode), per j-tile: var-Z(jt) then pairs with
                    # mean-Z(jt) with ~2 j-tiles of availability margin
                    # (a paired-2-jt variant starved the var slots and
                    # stalled the PE ~410ns every other j-tile-pair).
                    nc.vector.tensor_scalar(
                        out=ev_t[:, o : o + ROWS].bitcast(i16),
                        in0=em_t[:, o : o + ROWS].bitcast(i16),
                        scalar1=C_POW, scalar2=B_POW,
                        op0=Alu.mult, op1=Alu.add,
                    )
                    pending.extend(((jt, 0), (jt, 1)))
                if jt + 3 < NJT:
                    grams.append(emit_gram(jt + 3))

                # mean-Z for this j-tile (PE column groups 0-1), each half
                # followed by a pending var-Z half so the two Z streams sit
                # adjacent in the PE queue and overlap in disjoint col groups.
                vb = slice(jt * DY, (jt + 1) * DY)
                for h in range(2):
                    nc.tensor.matmul(
                        pzh[h][0:64, :], svm[:, vb],
                        em_t[:, o + h * H : o + (h + 1) * H],
                        start=(jt == 0), stop=(jt == NJT - 1),
                    )
                    if pending:
                        emit_var_z(*pending.pop(0))
            while pending:
                emit_var_z(*pending.pop(0))

            # Tail: one [128, 512] fp32->fp16 copy per i-half (ScalarE takes
            # h0 the moment pzh[0]'s last matmul stops, VectorE h1), then one
            # out-DMA per hardware queue.
            szT = data.tile([128, ROWS], f16)
            for h, (ceng, q) in enumerate(
                [(nc.scalar, nc.sync), (nc.vector, nc.gpsimd)]
            ):
                hs = slice(h * H, (h + 1) * H)
                if ceng is nc.scalar:
                    ceng.copy(szT[:, hs], pzh[h][:, :])
                else:
                    ceng.tensor_copy(szT[:, hs], pzh[h][:, :])
                q.dma_start(out=zT[:, hs], in_=szT[:, hs])

    nc.finalize()
    return nc


def _get_nc():
    if "nc" not in _CACHE:
        _CACHE["nc"] = _build_nc()
    return _CACHE["nc"]


def prep_inputs(X_mu, Y_eta, Y_mean, Y_var, V_mean, V_var):
    """Host-side prep: layouts, norms, prescaled V.  Returns (in_maps, e_m, e_v, ymv)."""
    X_mu, Y_eta, Y_mean, Y_var, V_mean, V_var = (
        np.asarray(a, dtype=np.float32)
        for a in (X_mu, Y_eta, Y_mean, Y_var, V_mean, V_var)
    )
    ymv = (Y_mean.astype(np.float64) + Y_var.astype(np.float64)).astype(np.float32)
    # fp16 features: PE products of fp16 inputs are exact in the fp32 PSUM
    # accumulation, so deriving the row norms from the QUANTIZED features
    # keeps k = e_i e_j exp(G/l^2) consistent.
    Xm = np.concatenate([X_mu, ymv], axis=1).astype(np.float32).astype(np.float16)
    f = (VAR_EPS * Y_eta[::-1].astype(np.float64)).astype(np.float16)  # [N, 64]

    rn_m = np.sum(Xm.astype(np.float64) ** 2, axis=1)                # [N]
    rn_v = rn_m + np.sum(f.astype(np.float64) ** 2, axis=1)

    e_m = np.exp(-rn_m / (2.0 * L_MEAN * L_MEAN))                    # fp64 [N]
    e_v = np.exp(-rn_v / (2.0 * L_VAR * L_VAR))

    # prescaled weights: V'' = e_j * V[j]
    Vm_p = (e_m[:, None] * V_mean.astype(np.float64)).astype(np.float16)
    Vv_p = (e_v[:, None] * V_var.astype(np.float64)).astype(np.float16)

    def v_tiles(Vp):
        # [128, jt*64+d] tile layout
        return np.ascontiguousarray(
            Vp.reshape(NJT, 128, DY).transpose(1, 0, 2).reshape(128, NJT * DY)
        )

    in_maps = []
    for c in range(NCORES):
        # j-rotation: roll rows so this core's own rows are j-tiles 0..7
        # (the kernel uses sxm[:, 0:ROWS] as both the Gram moving operand
        # and the first 8 j-tiles' weights -- one DMA instead of two).
        r = c * ROWS
        Xc = np.roll(Xm, -r, axis=0)
        in_maps.append(dict(
            xmT_all=np.ascontiguousarray(Xc.T),
            vm=v_tiles(np.roll(Vm_p, -r, axis=0)),
            vv=v_tiles(np.roll(Vv_p, -r, axis=0)),
        ))
    return in_maps, e_m, e_v, ymv


def postprocess(results, e_m, e_v, ymv):
    """Gather per-core z^T outputs and apply the e_i row factors + Y terms."""
    out = ymv.astype(np.float64).copy()
    sm = 4.0                      # undo the -ln4 shift in em
    sv = 4.0 ** C_POW             # undo the -c*ln4 shift in ev
    for c in range(NCORES):
        rs = slice(c * ROWS, (c + 1) * ROWS)
        zt = results[c]["zT"].astype(np.float64)  # [128, ROWS]
        out[rs] += (sm * e_m[rs])[:, None] * zt[0:64].T
        out[rs] += (sv * e_v[rs])[:, None] * zt[64:128].T
    return out.astype(np.float32)


def kernel(X_mu, Y_eta, Y_mean, Y_var, V_mean, V_var):
    from concourse.bass_utils import run_bass_kernel_spmd

    nc = _get_nc()
    in_maps, e_m, e_v, ymv = prep_inputs(X_mu, Y_eta, Y_mean, Y_var, V_mean, V_var)
    res = run_bass_kernel_spmd(nc, in_maps, core_ids=list(range(NCORES)))
    return postprocess(res.results, e_m, e_v, ymv)


# revision 30
# speedup vs baseline: 1.0465x; 1.0281x over previous
"""CondTransport kernel for Trainium2 (8 NeuronCores, row-parallel).

Math: Z = Y_mean + Y_var + k_mean @ V_mean + k_var @ V_var, where
k(X, X) = exp(-||x_i - x_j||^2 / (2 l^2)) are 8192x8192 RBF Gram matrices
over X_mean = [X_mu, Y_mean+Y_var] (96-d, l=7) and
X_var = [X_mu, 0.01*flip(Y_eta), Y_mean+Y_var] (160-d, l=9).

S-form factorization used on device:
  k[i,j] = e_i * e_j * exp(G_ij / l^2),   G = X X^T,  e_i = exp(-rn_i/(2 l^2))
so  Z[i] = e_i * sum_j exp(G_ij/l^2) (e_j V[j]).  The e_j factor is folded
into the V weights on the host, the e_i factor into the host epilogue, and
the device computes pure exp(G * scale + const).

Engine split (the v1 steady state was ScalarE-bound at 1087 ns/jt = 64
gapless ACTIVATE exps; v2+ rebalances):
  - em tiles are produced by TWO engines: ScalarE computes exact ACT-exp
    for 2/3 of j-tiles; the DVE computes the other 1/3 via a Schraudolph
    int16-bitcast exp (bits16(em) ~= A*G + B, one fp32-in tensor_scalar
    per tile, max value error ~3.1%, end-to-end contribution <1e-3).
  - VectorE derives the VAR map from em with the log-domain bitcast pow:
    bits16(em^c) ~= c*bits16(em) + (1-c)*15360 + B_ADJ  (c = 49/81), one
    int16 4x-mode tensor_scalar per j-tile (per-tile so var-Z(jt) pairs
    with mean-Z(jt) and nothing drains unpaired at the end).
  - PE: every matmul is M=64 into a PSUM partition half (Gram j-halves
    0:64/64:128 of pg; mean-Z 0:64 / var-Z 64:128 of pz), alternating the
    two PE column-group pairs, so consecutive ops execute CONCURRENTLY in
    disjoint column groups and every LDWEIGHTS hides under the other
    group's running matmul.  Steady state ~867 ns/j-tile vs the 854ns
    4x512-cycle floor (the M=128 Gram form paid ~240ns/jt of exposed
    weight-load latency).
  - j-ROTATION: per-core inputs are host-rolled so this core's own 1024
    rows are j-tiles 0..7; the Gram moving operand IS sxm[:, 0:1024] (no
    duplicate own-rows DMA, shortest critical path to the first Gram).
  - Startup: memsets + ACT-table-load + 5 warm-up matmuls cover the
    ~7.9..10us window until the first operands land (the NEFF preamble
    alone is ~6.4us; the PE p-state reaches 2.4GHz ~10-14us after first
    activity).  DMAs issue ONLY from sync/gpsimd (+1 early scalar DMA):
    descriptor-gen costs the issuing engine ~0.8us per DMA, so Scalar/
    Vector must stay DMA-free for the exp work.
  - Tail: pz is split per i-half into single-bank PSUM tiles; each half's
    fp32->fp16 copy (ScalarE h0 / VectorE h1) fires the moment its last
    matmul stops, then one out-DMA per queue.  The last two j-tiles'
    em/ev are produced in halves on both engines to shorten the end chain
    (em -> ev -> var-Z -> copy -> out-DMA).  Writeback in fp16 (fp16
    rounding of Z is ~5e-4 relative, negligible).

Sharding: rows i split 1024-per-core (8 cores); each core holds full X.
"""

import numpy as np

N = 8192
DX = 32
DY = 64
NCORES = 8
ROWS = N // NCORES          # 1024 rows per core
NJT = N // 128              # 64 j-tiles
DM = DX + DY                # 96 mean features
L_MEAN = 7.0
L_VAR = 9.0
VAR_EPS = 0.01

C_POW = (L_MEAN * L_MEAN) / (L_VAR * L_VAR)      # 49/81
SHIFT = float(np.log(4.0))                        # em = exp(G/49 - ln4)
B_POW = (1.0 - C_POW) * 15360.0 - 18.0            # fastpow offset (tuned)

LN2 = float(np.log(2.0))
INV2LM = 1.0 / (L_MEAN * L_MEAN)
A_EXP = 1024.0 * INV2LM / LN2                     # Schraudolph slope
B_EXP = 1024.0 * (15.0 - SHIFT / LN2) - 46.0      # Schraudolph offset (tuned)

_CACHE = {}


def _build_nc():
    import concourse.mybir as mybir
    import concourse.tile as tile
    from concourse import bacc

    f32 = mybir.dt.float32
    f16 = mybir.dt.float16
    i16 = mybir.dt.int16
    Exp = mybir.ActivationFunctionType.Exp
    Alu = mybir.AluOpType

    nc = bacc.Bacc(None, target_bir_lowering=False)

    # Per-core inputs are j-ROTATED by the core's row offset (host-side
    # np.roll), so each core's own rows ARE j-tiles 0..7: the Gram moving
    # operand is sxm[:, 0:ROWS] and no separate own-rows DMA is needed.
    xmT_all = nc.declare_dram_parameter("xmT_all", [DM, N], f16, isOutput=False)
    vm = nc.declare_dram_parameter("vm", [128, NJT * DY], f16, isOutput=False)
    vv = nc.declare_dram_parameter("vv", [128, NJT * DY], f16, isOutput=False)
    zT = nc.declare_dram_parameter("zT", [128, ROWS], f16, isOutput=True)

    with tile.TileContext(nc) as tc:
        with (
            tc.tile_pool(name="data", bufs=1) as data,
            tc.tile_pool(name="etiles", bufs=3) as etiles,
            tc.tile_pool(name="psg", bufs=3, space="PSUM") as psg,
            tc.tile_pool(name="psz", bufs=1, space="PSUM") as psz,
        ):
            sxm = data.tile([DM, N], f16)
            svm = data.tile([128, NJT * DY], f16)
            svv = data.tile([128, NJT * DY], f16)
            warm = data.tile([DM, 512], f16)
            bias_t = data.tile([128, 1], f32)
            dummy = data.tile([128, 1], f16)

            H = ROWS // 2  # 512-wide halves (PSUM bank / moving-op limit)
            # Z^T accumulators, ONE PSUM BANK PER i-HALF (partitions 0:64
            # mean, 64:128 var).  Separate tiles per half so each half's
            # PSUM->SBUF copy fires as soon as ITS last matmul stops (a
            # single [128,1024] tile serialized the copies behind the very
            # last var matmul).
            pzh = [psz.tile([128, H], f32, name=f"pz{h}") for h in range(2)]

            # --- t=0 work that runs during the DMA window -----------------
            # memsets on GpSimd (starts immediately), ACT table load early,
            # and 5 serial warm-up matmuls that keep the PE busy (p-state
            # ramping toward 2.4 GHz) until the first real Gram's operands
            # land (~10.3us; engine-start alone is ~7.2us).
            nc.gpsimd.memset(warm, 0.0)
            nc.gpsimd.memset(bias_t, -SHIFT)
            nc.scalar.activation(dummy, bias_t, Exp, bias=bias_t[:, :], scale=1.0)
            for w in range(5):
                nc.tensor.matmul(
                    pzh[0][:, :], warm[:, 0:128], warm[:, :], start=True, stop=True,
                )

            # --- DMA-in.  Head: the own block [0:1024] (j-tiles 0..7 AND
            # the Gram moving operand, thanks to the rotation) split across
            # sync/gpsimd; j-tiles 8..11 on ONE early scalar-queue DMA (the
            # Scalar engine must otherwise stay DMA-free: queued DMA issue
            # ahead of the EXPs delays them by ~0.8us each).
            nc.sync.dma_start(out=sxm[:, 0:512], in_=xmT_all[:, 0:512])
            nc.gpsimd.dma_start(out=sxm[:, 512:1024], in_=xmT_all[:, 512:1024])
            nc.scalar.dma_start(out=sxm[:, 1024:1536], in_=xmT_all[:, 1024:1536])
            # Rest interleaved in consumption order (V chunk k feeds the Z
            # matmuls of j-tiles 8k..8k+7; X feeds Grams ~3 j-tiles ahead).
            VCH = (NJT * DY) // 8
            items = []
            for k in range(8):
                items.append((svm, vm, slice(k * VCH, (k + 1) * VCH)))
                items.append((svv, vv, slice(k * VCH, (k + 1) * VCH)))
                cs, ce = 1536 + k * 1024, min(1536 + (k + 1) * 1024, N)
                if cs < N:
                    items.append((sxm, xmT_all, slice(cs, ce)))
            for i, (dst, src, sl) in enumerate(items):
                q = nc.sync if i % 2 == 0 else nc.gpsimd
                q.dma_start(out=dst[:, sl], in_=src[:, sl])

            def emit_gram(jt):
                """G(jt) [128 j x ROWS i] fp32 into a rotating PSUM slot.

                Emitted as FOUR M=64 matmuls alternating between the two PE
                column-group pairs (out partitions 0:64 vs 64:128), so every
                weight load lands in one group while the other group's matmul
                streams -- no exposed LDWEIGHTS latency (the full-width M=128
                form paid ~240ns/jt of weight-load stalls).
                """
                jb = jt * 128
                pg = psg.tile([128, ROWS], f32, tag="pg", bufs=3, name=f"pg{jt}")
                for h in range(2):
                    hs = slice(h * H, (h + 1) * H)
                    nc.tensor.matmul(
                        pg[0:64, hs], sxm[:, jb : jb + 64], sxm[:, hs],
                        start=True, stop=True,
                    )
                    nc.tensor.matmul(
                        pg[64:128, hs], sxm[:, jb + 64 : jb + 128], sxm[:, hs],
                        start=True, stop=True,
                    )
                return pg

            # Main loop.  em/ev tiles span 4 j-tiles (SBUF layout only); the
            # ev fastpow runs PER j-tile so var-Z(jt) pairs with mean-Z(jt)
            # in the same slot -- zero end-of-loop drain (the 4-tile-grouped
            # ev lagged ~2 groups and left ~3.3us of unpaired var-Z at the
            # end, which also blocked the PSUM->SBUF copies).
            GRP = 4
            grams = [emit_gram(0), emit_gram(1), emit_gram(2)]
            em_t = ev_t = None
            ev_tiles = {}
            pending = []  # deferred var-Z j-tiles, interleaved with later mean-Z

            def emit_var_z(jv, h):
                """var-Z half (PE column groups 2-3, concurrent with mean-Z)."""
                vbv = slice(jv * DY, (jv + 1) * DY)
                ov = (jv % GRP) * ROWS
                nc.tensor.matmul(
                    pzh[h][64:128, :], svv[:, vbv],
                    ev_tiles[jv // GRP][:, ov + h * H : ov + (h + 1) * H],
                    start=(jv == 0), stop=(jv == NJT - 1),
                )

            for jt in range(NJT):
                g, o = jt // GRP, (jt % GRP) * ROWS
                if jt % GRP == 0:
                    em_t = etiles.tile([128, GRP * ROWS], f16, tag="em", name=f"em{g}")
                    ev_t = etiles.tile([128, GRP * ROWS], f16, tag="ev", name=f"ev{g}")
                    ev_tiles[g] = ev_t
                pg = grams[jt]
                if jt >= NJT - 2:
                    # Last two j-tiles sit on the kernel's end chain
                    # (em -> ev -> var-Z -> copy -> out-DMA): produce their
                    # em/ev in HALVES on both engines concurrently.
                    nc.scalar.activation(
                        em_t[:, o : o + H], pg[:, 0:H], Exp,
                        bias=bias_t[:, :], scale=INV2LM,
                    )
                    nc.vector.tensor_scalar(
                        out=em_t[:, o + H : o + ROWS].bitcast(i16),
                        in0=pg[:, H:ROWS],
                        scalar1=A_EXP, scalar2=B_EXP,
                        op0=Alu.mult, op1=Alu.add,
                    )
                    for fh in (1, 0):
                        fs = slice(o + fh * H, o + (fh + 1) * H)
                        nc.vector.tensor_scalar(
                            out=ev_t[:, fs].bitcast(i16),
                            in0=em_t[:, fs].bitcast(i16),
                            scalar1=C_POW, scalar2=B_POW,
                            op0=Alu.mult, op1=Alu.add,
                        )
                    pending.extend(((jt, 0), (jt, 1)))
                else:
                    if jt % 3 == 1:
                        # DVE Schraudolph exp: bits16(em) = A*G + B (fp32-in)
                        nc.vector.tensor_scalar(
                            out=em_t[:, o : o + ROWS].bitcast(i16),
                            in0=pg[:, :],
                            scalar1=A_EXP, scalar2=B_EXP,
                            op0=Alu.mult, op1=Alu.add,
                        )
                    else:
                        nc.scalar.activation(
                            em_t[:, o : o + ROWS], pg[:, :], Exp,
                            bias=bias_t[:, :], scale=INV2LM,
                        )
                    # fastpow: ev = em^(49/81) via int16 bitcast (DVE 4x
                    # mode), per j-tile: var-Z(jt) then pairs with
                    # mean-Z(jt) with ~2 j-tiles of availability margin
                    # (a paired-2-jt variant starved the var slots and
                    # stalled the PE ~410ns every other j-tile-pair).
                    nc.vector.tensor_scalar(
                        out=ev_t[:, o : o + ROWS].bitcast(i16),
                        in0=em_t[:, o : o + ROWS].bitcast(i16),
                        scalar1=C_POW, scalar2=B_POW,
                        op0=Alu.mult, op1=Alu.add,
                    )
                    pending.extend(((jt, 0), (jt, 1)))
                if jt + 3 < NJT:
                    grams.append(emit_gram(jt + 3))

                # mean-Z for this j-tile (PE column groups 0-1), each half
                # followed by a pending var-Z half so the two Z streams sit
                # adjacent in the PE queue and overlap in disjoint col groups.
                vb = slice(jt * DY, (jt + 1) * DY)
                for h in range(2):
                    nc.tensor.matmul(
                        pzh[h][0:64, :], svm[:, vb],
                        em_t[:, o + h * H : o + (h + 1) * H],
                        start=(jt == 0), stop=(jt == NJT - 1),
                    )
                    if pending:
                        emit_var_z(*pending.pop(0))
            while pending:
                emit_var_z(*pending.pop(0))

            # Tail: one [128, 512] fp32->fp16 copy per i-half (ScalarE takes
            # h0 the moment pzh[0]'s last matmul stops, VectorE h1), then one
            # out-DMA per hardware queue.  Both out-DMAs ride HWDGE queues
            # (sync + scalar): gpsimd's SWDGE would add ~0.66us of Q7
            # descriptor-gen right on the end chain, and the Scalar engine
            # is idle once its h0 copy is done.
            szT = data.tile([128, ROWS], f16)
            for h, (ceng, q) in enumerate(
                [(nc.scalar, nc.sync), (nc.vector, nc.scalar)]
            ):
                hs = slice(h * H, (h + 1) * H)
                if ceng is nc.scalar:
                    ceng.copy(szT[:, hs], pzh[h][:, :])
                else:
                    ceng.tensor_copy(szT[:, hs], pzh[h][:, :])
                q.dma_start(out=zT[:, hs], in_=szT[:, hs])

    nc.finalize()
    return nc


def _get_nc():
    if "nc" not in _CACHE:
        _CACHE["nc"] = _build_nc()
    return _CACHE["nc"]


def prep_inputs(X_mu, Y_eta, Y_mean, Y_var, V_mean, V_var):
    """Host-side prep: layouts, norms, prescaled V.  Returns (in_maps, e_m, e_v, ymv)."""
    X_mu, Y_eta, Y_mean, Y_var, V_mean, V_var = (
        np.asarray(a, dtype=np.float32)
        for a in (X_mu, Y_eta, Y_mean, Y_var, V_mean, V_var)
    )
    ymv = (Y_mean.astype(np.float64) + Y_var.astype(np.float64)).astype(np.float32)
    # fp16 features: PE products of fp16 inputs are exact in the fp32 PSUM
    # accumulation, so deriving the row norms from the QUANTIZED features
    # keeps k = e_i e_j exp(G/l^2) consistent.
    Xm = np.concatenate([X_mu, ymv], axis=1).astype(np.float32).astype(np.float16)
    f = (VAR_EPS * Y_eta[::-1].astype(np.float64)).astype(np.float16)  # [N, 64]

    rn_m = np.sum(Xm.astype(np.float64) ** 2, axis=1)                # [N]
    rn_v = rn_m + np.sum(f.astype(np.float64) ** 2, axis=1)

    e_m = np.exp(-rn_m / (2.0 * L_MEAN * L_MEAN))                    # fp64 [N]
    e_v = np.exp(-rn_v / (2.0 * L_VAR * L_VAR))

    # prescaled weights: V'' = e_j * V[j]
    Vm_p = (e_m[:, None] * V_mean.astype(np.float64)).astype(np.float16)
    Vv_p = (e_v[:, None] * V_var.astype(np.float64)).astype(np.float16)

    def v_tiles(Vp):
        # [128, jt*64+d] tile layout
        return np.ascontiguousarray(
            Vp.reshape(NJT, 128, DY).transpose(1, 0, 2).reshape(128, NJT * DY)
        )

    in_maps = []
    for c in range(NCORES):
        # j-rotation: roll rows so this core's own rows are j-tiles 0..7
        # (the kernel uses sxm[:, 0:ROWS] as both the Gram moving operand
        # and the first 8 j-tiles' weights -- one DMA instead of two).
        r = c * ROWS
        Xc = np.roll(Xm, -r, axis=0)
        in_maps.append(dict(
            xmT_all=np.ascontiguousarray(Xc.T),
            vm=v_tiles(np.roll(Vm_p, -r, axis=0)),
            vv=v_tiles(np.roll(Vv_p, -r, axis=0)),
        ))
    return in_maps, e_m, e_v, ymv


def postprocess(results, e_m, e_v, ymv):
    """Gather per-core z^T outputs and apply the e_i row factors + Y terms."""
    out = ymv.astype(np.float64).copy()
    sm = 4.0                      # undo the -ln4 shift in em
    sv = 4.0 ** C_POW             # undo the -c*ln4 shift in ev
    for c in range(NCORES):
        rs = slice(c * ROWS, (c + 1) * ROWS)
        zt = results[c]["zT"].astype(np.float64)  # [128, ROWS]
        out[rs] += (sm * e_m[rs])[:, None] * zt[0:64].T
        out[rs] += (sv * e_v[rs])[:, None] * zt[64:128].T
    return out.astype(np.float32)


def kernel(X_mu, Y_eta, Y_mean, Y_var, V_mean, V_var):
    from concourse.bass_utils import run_bass_kernel_spmd

    nc = _get_nc()
    in_maps, e_m, e_v, ymv = prep_inputs(X_mu, Y_eta, Y_mean, Y_var, V_mean, V_var)
    res = run_bass_kernel_spmd(nc, in_maps, core_ids=list(range(NCORES)))
    return postprocess(res.results, e_m, e_v, ymv)
